# revision 22
# baseline (speedup 1.0000x reference)
"""Trainium2 Bass kernel: Ernie4.5 VisionAttention (varlen attention, 4x512
segments, 16 heads x 80 dim, embed 1280).

Sharding: 8 cores = 2 segment-groups (2x512 tokens each) x 4 head-groups
(4 heads each). Tensor-parallel over heads (qkv column-shard, proj row-shard),
data-parallel over segment pairs. No collectives: per-core proj partials are
summed on the host.

v2 schedule: the input stream (x|wqk|wv) is consumed K-OUTER while it lands --
8 PSUM banks accumulate seg-0's first four qk tiles plus its four v chunks,
one 128-row K chunk per arriving stream chunk, so the PE is fed at DMA rate
from the first chunk on. After stream-in everything is resident and the
pipeline runs seg-0 attention under seg-1's projection, all of seg-0's output
projection inside that window, and a split output projection for seg-1: the
proj weight is repacked host-side so head 3 lives alone in K-chunk 2, letting
chunks 0-1 of every seg-1 out-proj group pre-accumulate during the last
attention and the final chunk read head 3's normalized context directly
(no repack DMA on the tail critical path).

Heads are interleaved in the packed qk projection [q0 k0 q1 k1 ...]; the
rotary swap-half is a matmul against a packed +-1 permutation (fp8 weights,
exact). The softmax denominator rides partition 96 of the ctx matmul via a
ones column in the 97-wide v blocks; 1/den is broadcast with a K=1 matmul
(tile_position=(96,0)) and applied directly to the ctx PSUM (no intermediate
evict). DMA dispatch is spread across sync (loads/stores), gpsimd (qk unpack)
and scalar (ctx repack) queues.

Compute dtype: bf16 operands, fp32 PSUM accumulation.
"""

import sys

if "/opt/trn_rl_repo" not in sys.path:
    sys.path.insert(0, "/opt/trn_rl_repo")

import numpy as np
import ml_dtypes

BF = ml_dtypes.bfloat16

EMBED = 1280
HEADS = 16
HD = 80          # head dim
RH = 40          # rotary half
SEQ = 2048
SEGLEN = 512
N_CORES = 8
HPC = 4          # heads per core
TOK = 1024       # tokens per core (2 segments)
NSEG = 2
NUNITS = 2 * HPC # unit 2j = q of head j, unit 2j+1 = k of head j
VW = 97          # v block width per head in SBUF (80 v + 16 pad + 1 ones col)
VTOT = HPC * VW  # 388 (sbuf layout)
VC = HPC * HD    # 320 compact v weight width (streamed; scattered on evict)
SW = TOK + NUNITS * HD + VC  # stream row: xt | wqk | wv = 1024+640+320
SCALE = HD ** -0.5
KCH = EMBED // 128  # 10

_CACHE = {}

# unpack pieces: packed row 80u+d lives in tile t=(80u+d)//128; piece list
# per packed tile t: (unit, unit_row_offset, tile_row_offset, length)
UNPACK_PIECES = {t: [] for t in range(5)}
for _u in range(NUNITS):
    _a = HD * _u
    while _a < HD * (_u + 1):
        _t = _a // 128
        _b = min(HD * (_u + 1), 128 * (_t + 1))
        UNPACK_PIECES[_t].append((_u, _a - HD * _u, _a - 128 * _t, _b - _a))
        _a = _b

# pi-swap source blocks per packed tile t (rows shift by +-40 inside each
# 80-row unit => sources span tiles t-1..t+1)
PI_BLOCKS = {0: [0, 1], 1: [0, 1, 2], 2: [1, 2, 3], 3: [2, 3, 4], 4: [3, 4]}

# ctx repack for the out-proj: head j of a segment lands at wp_cat row
# 80j (j<3) / 256 (j=3), i.e. head 3 is alone in K-chunk 2.
# (unit-row-offset, dst tile, dst row, length) pieces per head; head 3 of
# seg 1 is consumed directly from its ctxn tile (no repack).
CTXP_PIECES = {
    0: [(0, 0, 0, 80)],
    1: [(0, 0, 80, 48), (48, 1, 0, 32)],
    2: [(0, 1, 32, 80)],
    3: [(0, 2, 0, 80)],
}


def _build_program():
    import concourse.tile as tile
    from concourse import bacc, mybir

    f32 = mybir.dt.float32
    f16 = mybir.dt.float16
    bf16 = mybir.dt.bfloat16
    fp8 = mybir.dt.float8e4
    AF = mybir.ActivationFunctionType
    ALU = mybir.AluOpType

    nc = bacc.Bacc("TRN2", target_bir_lowering=False, debug=False,
                   num_devices=N_CORES)

    stream_d = nc.dram_tensor("stream", [EMBED, SW], bf16,
                              kind="ExternalInput").ap()
    vpat_d = nc.dram_tensor("vpat", [128, VC], bf16,
                            kind="ExternalInput").ap()
    wp_d = nc.dram_tensor("wp", [128, 3 * EMBED], bf16,
                          kind="ExternalInput").ap()
    bias_d = nc.dram_tensor("biasqk", [128, 5], f32, kind="ExternalInput").ap()
    cos_d = nc.dram_tensor("cosm", [NUNITS * HD, TOK], bf16,
                           kind="ExternalInput").ap()
    sin_d = nc.dram_tensor("sinm", [NUNITS * HD, TOK], bf16,
                           kind="ExternalInput").ap()
    pit_d = nc.dram_tensor("pit", [NUNITS * HD, NUNITS * HD], fp8,
                           kind="ExternalInput").ap()
    out_d = nc.dram_tensor("outT", [EMBED, TOK], f16, kind="ExternalOutput").ap()

    def load_grouped(dst_tile, src_ap, col_w, groups, eng=None):
        for e0, e1 in groups:
            src = src_ap[128 * e0:128 * e1, :].rearrange(
                "(e p) t -> p e t", p=128)
            dst = dst_tile[:, col_w * e0:col_w * e1].rearrange(
                "p (e t) -> p e t", t=col_w)
            (eng or nc.sync).dma_start(dst, src)

    with tile.TileContext(nc) as tc:
        with tc.tile_pool(name="persist", bufs=1) as P:
            # ---- batched persistent loads, in consumption order: the whole
            # stream first (phase A is paced by it), then rotary constants,
            # then late-needed cos/sin tails and the out-proj weight ----
            str_sb = P.tile([128, KCH * SW], bf16, name="str_sb", tag="str")
            cos_sb = P.tile([128, 5 * TOK], bf16, name="cos_sb", tag="cos")
            sin_sb = P.tile([128, 5 * TOK], bf16, name="sin_sb", tag="sin")
            pit_sb = P.tile([128, 5 * NUNITS * HD], fp8, name="pit_sb",
                            tag="pit")
            bias_sb = P.tile([128, 5], f32, name="biasqk_sb", tag="biasqk")
            vpat_sb = P.tile([128, VC], bf16, name="vpat_sb", tag="vpat")
            # a single DMA transfer runs on one ring at ~70GB/s; aggregate
            # bandwidth needs PARALLEL transfers. Split the critical first
            # chunks column-wise and dispatch across four engine queues so
            # the first chunk lands ~6us instead of ~11us.
            def load_piece(e0, e1, c0, c1, eng):
                src = stream_d[128 * e0:128 * e1, c0:c1].rearrange(
                    "(e p) t -> p e t", p=128)
                dst = str_sb[:, :].rearrange(
                    "p (e t) -> p e t", t=SW)[:, e0:e1, c0:c1]
                eng.dma_start(dst, src)

            Q, H = SW // 4, SW // 2
            load_piece(0, 1, 0, Q, nc.sync)
            load_piece(0, 1, Q, 2 * Q, nc.scalar)
            load_piece(0, 1, 2 * Q, 3 * Q, nc.gpsimd)
            load_piece(0, 1, 3 * Q, SW, nc.sync)
            load_piece(1, 2, 0, H, nc.scalar)
            load_piece(1, 2, H, SW, nc.gpsimd)
            load_piece(2, 3, 0, H, nc.sync)
            load_piece(2, 3, H, SW, nc.scalar)
            load_piece(3, 4, 0, H, nc.sync)
            load_piece(3, 4, H, SW, nc.gpsimd)
            # chunks 4-9: two parallel rings each (a single ring moves only
            # ~40-70GB/s; per-chunk halves keep arrival ahead of the PE)
            for e in (4, 6):
                load_piece(e, e + 1, 0, H, nc.scalar)
                load_piece(e, e + 1, H, SW, nc.scalar)
            for e in (5, 7, 8, 9):
                load_piece(e, e + 1, 0, H, nc.sync)
                load_piece(e, e + 1, H, SW, nc.sync)
            load_grouped(pit_sb, pit_d, NUNITS * HD, [(0, 2)], eng=nc.scalar)
            load_grouped(cos_sb, cos_d, TOK, [(0, 2)], eng=nc.scalar)
            load_grouped(sin_sb, sin_d, TOK, [(0, 2)], eng=nc.scalar)
            nc.scalar.dma_start(bias_sb[:], bias_d[:])
            nc.scalar.dma_start(vpat_sb[:], vpat_d[:])
            wp_sb = P.tile([128, 3 * EMBED], bf16, name="wp_sb", tag="wp")
            # late loads (cos/sin/pit tails + wp) are not needed until after
            # stream-in; gate their transfers on stream chunk 6's arrival via
            # 1-element WAW writes into each destination so they don't steal
            # HBM bandwidth from the critical stream chunks
            LATE = [(pit_sb, pit_d, NUNITS * HD), (cos_sb, cos_d, TOK),
                    (sin_sb, sin_d, TOK)]
            gate_src = str_sb[0:1, SW * 6:SW * 6 + 1]
            for dst_t, _, cw in LATE:
                nc.vector.tensor_copy(dst_t[0:1, cw * 2:cw * 2 + 1], gate_src)
            nc.vector.tensor_copy(wp_sb[0:1, 0:1], gate_src)
            # gated dispatches ride the gpsimd queue, whose first real op
            # (the first rotary multiply) starts at stream-end anyway
            for dst_t, src_d, cw in LATE:
                load_grouped(dst_t, src_d, cw, [(2, 5)], eng=nc.gpsimd)
            nc.gpsimd.dma_start(wp_sb[:], wp_d[:])

            def xt(e):
                return str_sb[:, SW * e:SW * e + TOK]

            def wqk(e):
                o = SW * e + TOK
                return str_sb[:, o:o + NUNITS * HD]

            def wv(e):
                o = SW * e + TOK + NUNITS * HD
                return str_sb[:, o:o + VC]

            # persistent intermediates
            qkp_sb = [[None] * NSEG for _ in range(5)]
            qrot = [P.tile([HD, TOK], bf16, name=f"qrot{u}", tag=f"qrot{u}")
                    for u in range(NUNITS)]
            v_sb = [P.tile([128, VTOT], bf16, name=f"vsb{m}", tag=f"vsb{m}")
                    for m in range(TOK // 128)]
            # repacked ctx for the out-proj (head 3 of seg 1 is read directly
            # from its ctxn tile instead)
            ctxp = {(0, 0): None, (0, 1): None, (0, 2): None,
                    (1, 0): None, (1, 1): None}
            for (s_, c_) in list(ctxp):
                ctxp[(s_, c_)] = P.tile([128, SEGLEN], bf16,
                                        name=f"ctxp{c_}_{s_}",
                                        tag=f"ctxp{c_}_{s_}")
            o_sb = [P.tile([128, KCH * SEGLEN], f16, name=f"osb{s}",
                           tag=f"osb{s}") for s in range(NSEG)]
            scr = P.tile([128, 16], bf16, name="scr", tag="scr")

            # PSUM budget (8 banks): qk-proj, pi-swap and out-proj share a
            # 3-slot pool; v + 1/den broadcast 1; scores 2; ctx 2. Phase A
            # borrows all 8 for the K-outer stream consumption.
            with tc.tile_pool(name="ps_a", bufs=3, space="PSUM") as PSA, \
                 tc.tile_pool(name="ps_v", bufs=1, space="PSUM") as PSV, \
                 tc.tile_pool(name="ps_st", bufs=2, space="PSUM") as PST, \
                 tc.tile_pool(name="ps_ctx", bufs=2, space="PSUM") as PSC, \
                 tc.tile_pool(name="work", bufs=3) as W, \
                 tc.tile_pool(name="workd", bufs=6) as WD:

                ones80 = P.tile([128, HD], bf16, name="ones80", tag="ones80")
                nc.vector.memset(ones80[:], 1.0)
                # preload the ACT exp table while the engine is idle
                nc.scalar.activation(scr[:], ones80[:, 0:16], AF.Exp)
                # pad + ones columns of every v tile, once (strided memsets)
                for m_ in range(TOK // 128):
                    blk = v_sb[m_][:, :].rearrange("p (j c) -> p j c", c=VW)
                    nc.vector.memset(blk[:, :, HD:96], 0.0)
                    nc.vector.memset(blk[:, :, 96:97], 1.0)
                # zero the repack pad rows once (base partition must be
                # 32-aligned; rows 96-112 get overwritten by the j=2 piece)
                for s in range(NSEG):
                    nc.vector.memset(ctxp[(s, 1)][96:128, :], 0.0)

                est = {}    # (s, j) -> list of 4 exp'd score tiles
                ctxn_last = {}  # s -> ctxn tile of head 3 (seg-1 tail path)

                def qkproj(t, s):
                    sc = slice(SEGLEN * s, SEGLEN * (s + 1))
                    qk_ps = PSA.tile([128, SEGLEN], f32, name=f"qkps{t}_{s}",
                                     tag="mm512")
                    for e in range(KCH):
                        nc.tensor.matmul(qk_ps[:],
                                         wqk(e)[:, 128 * t:128 * (t + 1)],
                                         xt(e)[:, sc],
                                         start=(e == 0), stop=(e == KCH - 1))
                    qk_evict(t, s, qk_ps)

                def qk_evict(t, s, qk_ps):
                    q_sb = W.tile([128, SEGLEN], bf16, name=f"qsb{t}_{s}",
                                  tag="qsb", bufs=10)
                    nc.scalar.activation(q_sb[:], qk_ps[:], AF.Identity,
                                         bias=bias_sb[:, t:t + 1])
                    qkp_sb[t][s] = q_sb

                def v_evict(m, v_ps):
                    # scatter compact 80-col head blocks into the 97-stride
                    # v layout, adding the v-bias row pattern
                    dst = v_sb[m][:, :].rearrange("p (j c) -> p j c",
                                                  c=VW)[:, :, 0:HD]
                    nc.vector.scalar_tensor_tensor(
                        dst, v_ps[:].rearrange("p (j c) -> p j c", c=HD),
                        1.0, vpat_sb[:].rearrange("p (j c) -> p j c", c=HD),
                        ALU.mult, ALU.add)

                def vchunk(m, pool):
                    mc = slice(128 * m, 128 * (m + 1))
                    v_ps = pool.tile([128, VC], f32, name=f"vps{m}",
                                     tag={id(PST): "stps", id(PSC): "ctxps",
                                          id(PSV): "vps"}[id(pool)])
                    for e in range(KCH):
                        nc.tensor.matmul(v_ps[:], xt(e)[:, mc], wv(e),
                                         start=(e == 0), stop=(e == KCH - 1))
                    v_evict(m, v_ps)

                def rotary(tr, s):
                    sc = slice(SEGLEN * s, SEGLEN * (s + 1))
                    qsw_ps = PSA.tile([128, SEGLEN], f32, name=f"qsw{tr}_{s}",
                                      tag="mm512")
                    srcs = PI_BLOCKS[tr]
                    for i, tp in enumerate(srcs):
                        nc.tensor.matmul(qsw_ps[:],
                                         pit_sb[:, NUNITS * HD * tp + 128 * tr:
                                                NUNITS * HD * tp + 128 * (tr + 1)],
                                         qkp_sb[tp][s][:],
                                         start=(i == 0),
                                         stop=(i == len(srcs) - 1))
                    # t1 = q*cos on the (otherwise idle) gpsimd engine;
                    # t2 = swap(q)*sin reads PSUM so it stays on vector
                    t1 = W.tile([128, SEGLEN], bf16, name=f"t1_{tr}_{s}",
                                tag="t1", bufs=4)
                    nc.gpsimd.tensor_tensor(t1[:], qkp_sb[tr][s][:],
                                            cos_sb[:, TOK * tr + sc.start:
                                                   TOK * tr + sc.stop],
                                            ALU.mult)
                    t2 = W.tile([128, SEGLEN], bf16, name=f"t2_{tr}_{s}",
                                tag="t2", bufs=4)
                    nc.vector.tensor_tensor(t2[:], qsw_ps[:],
                                            sin_sb[:, TOK * tr + sc.start:
                                                   TOK * tr + sc.stop],
                                            ALU.mult)
                    rp = W.tile([128, SEGLEN], bf16, name=f"rotp{tr}_{s}",
                                tag="rotp", bufs=6)
                    nc.vector.tensor_tensor(rp[:], t1[:], t2[:], ALU.add)
                    for (u, po, toff, ln) in UNPACK_PIECES[tr]:
                        nc.gpsimd.dma_start(qrot[u][po:po + ln, sc],
                                            rp[toff:toff + ln, :])

                def scores(s, j):
                    sc = slice(SEGLEN * s, SEGLEN * (s + 1))
                    lst = []
                    for tkc in range(SEGLEN // 128):
                        kc = slice(SEGLEN * s + 128 * tkc,
                                   SEGLEN * s + 128 * (tkc + 1))
                        st_ps = PST.tile([128, SEGLEN], f32,
                                         name=f"st{j}_{s}_{tkc}", tag="stps")
                        nc.tensor.matmul(st_ps[:], qrot[2 * j + 1][:, kc],
                                         qrot[2 * j][:, sc],
                                         start=True, stop=True)
                        e_sb = WD.tile([128, SEGLEN], bf16,
                                       name=f"est{j}_{s}_{tkc}", tag="est",
                                       bufs=14)
                        nc.scalar.activation(e_sb[:], st_ps[:], AF.Exp)
                        lst.append(e_sb)
                    est[(s, j)] = lst

                ctx_pend = {}   # (s, j) -> (ctx_ps, rec_bf)

                def ctx_mm(s, j):
                    lst = est.pop((s, j))
                    ctx_ps = PSC.tile([128, SEGLEN], f32, name=f"ctxps{j}_{s}",
                                      tag="ctxps")
                    for tkc in range(SEGLEN // 128):
                        nc.tensor.matmul(ctx_ps[0:VW, :],
                                         v_sb[4 * s + tkc][:, VW * j:VW * (j + 1)],
                                         lst[tkc][:],
                                         start=(tkc == 0), stop=(tkc == 3))
                    # den accumulated into partition 96 (ones column of the
                    # v block). Full-width reciprocal: DVE cost is per-lane
                    # over columns so 128 partitions cost the same as 1, and
                    # the custom-DVE op only works at base partition 0; ACT
                    # Ln/Exp alternatives thrash the activation table (~1.5us
                    # ACT_TABLE_LOAD per Exp<->Ln switch, measured).
                    rec = WD.tile([128, SEGLEN], f32, name=f"rec{j}_{s}",
                                  tag="rec", bufs=2)
                    nc.vector.reciprocal_approx_fast(rec[:, :], ctx_ps[:, :])
                    rec_bf = WD.tile([128, SEGLEN], bf16,
                                     name=f"recbf{j}_{s}", tag="recbf",
                                     bufs=2)
                    nc.vector.tensor_copy(rec_bf[96:97, :], rec[96:97, :])
                    # evict the raw ctx now (releases the PSC bank; also a
                    # TensorTensor cannot read two PSUM banks at once)
                    ctx_sb = WD.tile([HD, SEGLEN], f32, name=f"ctxsb{j}_{s}",
                                     tag="ctxsb", bufs=2)
                    if j % 2 == 0:
                        nc.scalar.activation(ctx_sb[:], ctx_ps[0:HD, :],
                                             AF.Identity)
                    else:
                        nc.vector.tensor_copy(ctx_sb[:], ctx_ps[0:HD, :])
                    ctx_pend[(s, j)] = (ctx_sb, rec_bf)

                def ctx_post(s, j):
                    # emitted a beat later so the K=1 broadcast matmul (which
                    # waits on the reciprocal chain) doesn't head-of-line
                    # block ready matmuls in the PE FIFO
                    ctx_sb, rec_bf = ctx_pend.pop((s, j))
                    bc_ps = PSV.tile([HD, SEGLEN], f32, name=f"bc{j}_{s}",
                                     tag="vps")
                    nc.tensor.matmul(bc_ps[:], ones80[96:97, :],
                                     rec_bf[96:97, :], start=True, stop=True,
                                     tile_position=(96, 0))
                    ctxn = WD.tile([HD, SEGLEN], bf16, name=f"ctxn{j}_{s}",
                                   tag="ctxn", bufs=3)
                    nc.vector.tensor_tensor(ctxn[:], ctx_sb[:],
                                            bc_ps[:], ALU.mult)
                    if s == 1 and j == 3:
                        ctxn_last[s] = ctxn
                    else:
                        for (po, c, toff, ln) in CTXP_PIECES[j]:
                            nc.gpsimd.dma_start(
                                ctxp[(s, c)][toff:toff + ln, :],
                                ctxn[po:po + ln, :])

                def ctx(s, j):
                    ctx_mm(s, j)
                    ctx_post(s, j)

                def o_evict(e, s, o_ps):
                    oc = o_sb[s][:, SEGLEN * e:SEGLEN * (e + 1)]
                    if e % 2 == 0:
                        nc.vector.tensor_copy(oc, o_ps[:])
                    else:
                        nc.scalar.activation(oc, o_ps[:], AF.Identity)

                def op_c01(e, s, pool, tag):
                    o_ps = pool.tile([128, SEGLEN], f32, name=f"ops{e}_{s}",
                                     tag=tag)
                    for c in range(2):
                        nc.tensor.matmul(o_ps[:],
                                         wp_sb[:, EMBED * c + 128 * e:
                                               EMBED * c + 128 * (e + 1)],
                                         ctxp[(s, c)][:],
                                         start=(c == 0), stop=False)
                    return o_ps

                def op_c2(e, s, o_ps):
                    # K-chunk 2 is head 3 alone (80 rows); seg 1 reads the
                    # ctxn tile directly, seg 0 the repacked tile
                    rhs = (ctxn_last[1][0:HD, :] if s == 1
                           else ctxp[(0, 2)][0:HD, :])
                    nc.tensor.matmul(o_ps[0:128, :],
                                     wp_sb[0:HD, EMBED * 2 + 128 * e:
                                           EMBED * 2 + 128 * (e + 1)],
                                     rhs, start=False, stop=True)
                    o_evict(e, s, o_ps)

                def oproj(e, s):
                    o_ps = op_c01(e, s, PSA, "mm512")
                    op_c2(e, s, o_ps)

                def opair(e0, e1, s):
                    # both c01 groups first: gives the head-3 repack DMA (and
                    # the previous pair's evicts) a beat before c2 needs them
                    oa = op_c01(e0, s, PSA, "mm512")
                    ob = op_c01(e1, s, PSA, "mm512")
                    op_c2(e0, s, oa)
                    op_c2(e1, s, ob)

                def ostore(s, e0, e1):
                    src = o_sb[s][:, SEGLEN * e0:SEGLEN * e1].rearrange(
                        "p (e t) -> p e t", t=SEGLEN)
                    dst = out_d[128 * e0:128 * e1,
                                SEGLEN * s:SEGLEN * (s + 1)].rearrange(
                        "(e p) t -> p e t", p=128)
                    nc.sync.dma_start(dst, src)

                # ---- phase A: K-outer over the arriving stream. Four banks
                # accumulate seg-0's qk tiles 0-3, one K=128 slice per
                # arriving chunk; light enough that the PE tracks DMA pace
                # while cold, and the v chunks run warm in phase B ----
                qk_ko = [PSA.tile([128, SEGLEN], f32, name=f"koqk{t}",
                                  tag="mm512") for t in range(3)]
                qk_ko.append(PST.tile([128, SEGLEN], f32, name="koqk3",
                                      tag="stps"))
                s0 = slice(0, SEGLEN)
                for e in range(KCH):
                    for t in range(4):
                        nc.tensor.matmul(qk_ko[t][:],
                                         wqk(e)[:, 128 * t:128 * (t + 1)],
                                         xt(e)[:, s0],
                                         start=(e == 0), stop=(e == KCH - 1))
                for t in range(4):
                    qk_evict(t, 0, qk_ko[t])

                # ---- phase B: everything resident; seg-0 attention under
                # seg-1 projection, seg-0 out-proj inside that window ----
                vchunk(0, PST)
                vchunk(1, PSC)
                vchunk(2, PSC)
                vchunk(3, PSV)
                qkproj(4, 0)
                rotary(0, 0)
                rotary(1, 0)
                scores(0, 0)
                vchunk(4, PSC)
                ctx_mm(0, 0)
                rotary(2, 0)
                ctx_post(0, 0)
                scores(0, 1)
                vchunk(5, PSC)
                ctx_mm(0, 1)
                rotary(3, 0)
                ctx_post(0, 1)
                qkproj(0, 1)
                rotary(4, 0)
                scores(0, 2)
                vchunk(6, PST)
                ctx_mm(0, 2)
                qkproj(1, 1)
                ctx_post(0, 2)
                scores(0, 3)
                ctx_mm(0, 3)
                vchunk(7, PSV)
                ctx_post(0, 3)
                rotary(0, 1)
                qkproj(2, 1)
                opair(0, 1, 0)
                rotary(1, 1)
                scores(1, 0)
                ctx_mm(1, 0)
                qkproj(3, 1)
                ctx_post(1, 0)
                opair(2, 3, 0)
                rotary(2, 1)
                scores(1, 1)
                ctx_mm(1, 1)
                qkproj(4, 1)
                ctx_post(1, 1)
                opair(4, 5, 0)
                ostore(0, 0, 3)
                rotary(3, 1)
                rotary(4, 1)
                scores(1, 2)
                ctx_mm(1, 2)
                o6 = op_c01(6, 0, PSA, "mm512")
                ctx_post(1, 2)
                o7 = op_c01(7, 0, PSA, "mm512")
                op_c2(6, 0, o6)
                op_c2(7, 0, o7)
                opair(8, 9, 0)
                ostore(0, 3, 7)
                # ---- tail: pre-accumulate seg-1 out-proj chunks 0-1 during
                # the last attention; finish with the direct head-3 chunk ----
                scores(1, 3)
                oA = [op_c01(e, 1, PSA, "mm512") for e in range(3)]
                ctx_mm(1, 3)
                oB = [op_c01(e, 1, PST, "stps") for e in (3, 4)]
                oC = [op_c01(5, 1, PSC, "ctxps")]
                ctx_post(1, 3)
                ostore(0, 7, KCH)
                for e, o_ps in enumerate(oA + oB + oC):
                    op_c2(e, 1, o_ps)
                opair(6, 7, 1)
                ostore(1, 0, 5)
                opair(8, 9, 1)
                ostore(1, 5, KCH)

    nc.compile()
    return nc


def _prep_inputs(x, rotary_pos_emb, qkv_w, qkv_b):
    """Build per-core input shards (host-side layout/constant prep)."""
    from concourse import mybir

    x2 = np.asarray(x, np.float32).reshape(SEQ, EMBED)
    rope = np.asarray(rotary_pos_emb, np.float32)
    qkv_w = np.asarray(qkv_w, np.float32)
    qkv_b = np.asarray(qkv_b, np.float32)
    FP8 = mybir.dt.np(mybir.dt.float8e4)

    # packed rotary multipliers: packed row p = 80u + d -> r = d % 40
    r_idx = np.tile(np.arange(HD) % RH, NUNITS)      # [640]
    cos_full = np.cos(rope)[:, r_idx].T.astype(BF)   # [640, 2048]
    sin_full = np.sin(rope)[:, r_idx].T.astype(BF)

    # packed swap permutation (sign folded), block-diagonal per 80-row unit;
    # +-1 entries are exact in fp8
    D = NUNITS * HD
    Pi = np.zeros((D, D), np.float32)
    for u in range(NUNITS):
        o = HD * u
        for i in range(RH):
            Pi[o + i, o + i + RH] = -1.0
            Pi[o + i + RH, o + i] = 1.0
    pit = np.ascontiguousarray(Pi.T).astype(FP8)

    in_maps = []
    for c in range(N_CORES):
        sg, hg = divmod(c, HPC)
        toks = slice(TOK * sg, TOK * (sg + 1))
        heads = [HPC * hg + j for j in range(HPC)]

        xa = x2[toks].T                                   # [1280, 1024]

        # interleaved packing: unit 2j = q of head j, unit 2j+1 = k
        wqk = np.empty((EMBED, NUNITS * HD), np.float32)
        bias_flat = np.empty(NUNITS * HD, np.float32)
        for j, h in enumerate(heads):
            oq, ok = HD * 2 * j, HD * (2 * j + 1)
            wqk[:, oq:oq + HD] = qkv_w[HD * h:HD * (h + 1), :].T * SCALE
            bias_flat[oq:oq + HD] = qkv_b[HD * h:HD * (h + 1)] * SCALE
            ko = EMBED + HD * h
            wqk[:, ok:ok + HD] = qkv_w[ko:ko + HD, :].T
            bias_flat[ok:ok + HD] = qkv_b[ko:ko + HD]
        bias = np.ascontiguousarray(bias_flat.reshape(5, 128).T)

        # compact v weights (80 cols per head); the eviction scatters them
        # into 97-wide blocks whose col 96 is a memset ones column that makes
        # the ctx matmul accumulate the softmax denominator at partition 96
        wv = np.zeros((EMBED, VC), np.float32)
        vpat_row = np.zeros(VC, np.float32)
        for j, h in enumerate(heads):
            vo = 2 * EMBED + HD * h
            wv[:, HD * j:HD * (j + 1)] = qkv_w[vo:vo + HD, :].T
            vpat_row[HD * j:HD * (j + 1)] = qkv_b[vo:vo + HD]
        vpat = np.ascontiguousarray(np.broadcast_to(vpat_row, (128, VC)))

        stream = np.concatenate([xa, wqk, wv], axis=1)    # [1280, SW]

        # wp packed for the repacked-ctx out-proj: heads 0-2 stacked at rows
        # 0-239, head 3 alone at rows 256-335 (K-chunk 2), zeros elsewhere
        wp_cat = np.zeros((384, EMBED), np.float32)
        for j, h in enumerate(heads[:3]):
            wp_cat[HD * j:HD * (j + 1), :] = _PROJ_W[:, HD * h:HD * (h + 1)].T
        wp_cat[256:256 + HD, :] = _PROJ_W[:, HD * heads[3]:
                                          HD * (heads[3] + 1)].T
        wp = np.zeros((128, 3 * EMBED), np.float32)
        for c_ in range(3):
            wp[:, EMBED * c_:EMBED * (c_ + 1)] = wp_cat[128 * c_:128 * (c_ + 1)]

        in_maps.append({
            "stream": np.ascontiguousarray(stream).astype(BF),
            "vpat": vpat.astype(BF),
            "wp": np.ascontiguousarray(wp).astype(BF),
            "biasqk": bias,
            "cosm": np.ascontiguousarray(cos_full[:, toks]),
            "sinm": np.ascontiguousarray(sin_full[:, toks]),
            "pit": pit,
        })
    return in_maps


_PROJ_W = None


def run_on_device(inputs, trace=False, trace_cores=None):
    """Shard, run on 8 NeuronCores, gather. Returns (output, BassKernelResults)."""
    global _PROJ_W
    from concourse import bass_utils

    x = np.asarray(inputs["x"], np.float32)
    cu = np.asarray(inputs["cu_seqlens"]).tolist()
    assert cu == [0, 512, 1024, 1536, 2048], (
        f"kernel compiled for 4x512 segments, got cu_seqlens={cu}")
    assert x.shape == (SEQ, 1, EMBED)

    _PROJ_W = np.asarray(inputs["proj_w"], np.float32)
    in_maps = _prep_inputs(x, inputs["rotary_pos_emb"],
                           inputs["qkv_w"], inputs["qkv_b"])

    if "nc" not in _CACHE:
        _CACHE["nc"] = _build_program()
    nc = _CACHE["nc"]

    kw = {}
    if trace:
        kw = dict(trace=True, trace_cores=trace_cores or [0])
    res = bass_utils.run_bass_kernel_spmd(nc, in_maps,
                                          core_ids=list(range(N_CORES)), **kw)

    proj_b = np.asarray(inputs["proj_b"], np.float32)
    out = np.empty((SEQ, EMBED), np.float32)
    for sg in range(2):
        acc = res.results[HPC * sg + 0]["outT"].astype(np.float32)
        for hg in range(1, HPC):
            acc = acc + res.results[HPC * sg + hg]["outT"].astype(np.float32)
        out[TOK * sg:TOK * (sg + 1)] = acc.T
    out += proj_b
    return out.reshape(SEQ, 1, EMBED), res


def kernel(**inputs):
    out, _ = run_on_device(inputs, trace=False)
    return out


# revision 23
# speedup vs baseline: 1.2091x; 1.2091x over previous
"""Trainium2 Bass kernel: Ernie4.5 VisionAttention (varlen attention, 4x512
segments, 16 heads x 80 dim, embed 1280).

Sharding: 8 cores = 2 segment-groups (2x512 tokens each) x 4 head-groups
(4 heads each). Tensor-parallel over heads (qkv column-shard, proj row-shard),
data-parallel over segment pairs. No collectives: per-core proj partials are
summed on the host.

v2 schedule: the input stream (x|wqk|wv) is consumed K-OUTER while it lands --
8 PSUM banks accumulate seg-0's first four qk tiles plus its four v chunks,
one 128-row K chunk per arriving stream chunk, so the PE is fed at DMA rate
from the first chunk on. After stream-in everything is resident and the
pipeline runs seg-0 attention under seg-1's projection, all of seg-0's output
projection inside that window, and a split output projection for seg-1: the
proj weight is repacked host-side so head 3 lives alone in K-chunk 2, letting
chunks 0-1 of every seg-1 out-proj group pre-accumulate during the last
attention and the final chunk read head 3's normalized context directly
(no repack DMA on the tail critical path).

Heads are interleaved in the packed qk projection [q0 k0 q1 k1 ...]; the
rotary swap-half is a matmul against a packed +-1 permutation (fp8 weights,
exact). The softmax denominator rides partition 96 of the ctx matmul via a
ones column in the 97-wide v blocks; 1/den is broadcast with a K=1 matmul
(tile_position=(96,0)) and applied directly to the ctx PSUM (no intermediate
evict). DMA dispatch is spread across sync (loads/stores), gpsimd (qk unpack)
and scalar (ctx repack) queues.

Compute dtype: bf16 operands, fp32 PSUM accumulation.
"""

import sys

if "/opt/trn_rl_repo" not in sys.path:
    sys.path.insert(0, "/opt/trn_rl_repo")

import numpy as np
import ml_dtypes

BF = ml_dtypes.bfloat16

EMBED = 1280
HEADS = 16
HD = 80          # head dim
RH = 40          # rotary half
SEQ = 2048
SEGLEN = 512
N_CORES = 8
HPC = 4          # heads per core
TOK = 1024       # tokens per core (2 segments)
NSEG = 2
NUNITS = 2 * HPC # unit 2j = q of head j, unit 2j+1 = k of head j
VW = 97          # v block width per head in SBUF (80 v + 16 pad + 1 ones col)
VTOT = HPC * VW  # 388 (sbuf layout)
VC = HPC * HD    # 320 compact v weight width (streamed; scattered on evict)
SW = TOK + NUNITS * HD + VC  # stream row: xt | wqk | wv = 1024+640+320
SCALE = HD ** -0.5
KCH = EMBED // 128  # 10

_CACHE = {}

# unpack pieces: packed row 80u+d lives in tile t=(80u+d)//128; piece list
# per packed tile t: (unit, unit_row_offset, tile_row_offset, length)
UNPACK_PIECES = {t: [] for t in range(5)}
for _u in range(NUNITS):
    _a = HD * _u
    while _a < HD * (_u + 1):
        _t = _a // 128
        _b = min(HD * (_u + 1), 128 * (_t + 1))
        UNPACK_PIECES[_t].append((_u, _a - HD * _u, _a - 128 * _t, _b - _a))
        _a = _b

# pi-swap source blocks per packed tile t (rows shift by +-40 inside each
# 80-row unit => sources span tiles t-1..t+1)
PI_BLOCKS = {0: [0, 1], 1: [0, 1, 2], 2: [1, 2, 3], 3: [2, 3, 4], 4: [3, 4]}

# ctx repack for the out-proj: head j of a segment lands at wp_cat row
# 80j (j<3) / 256 (j=3), i.e. head 3 is alone in K-chunk 2.
# (unit-row-offset, dst tile, dst row, length) pieces per head; head 3 of
# seg 1 is consumed directly from its ctxn tile (no repack).
CTXP_PIECES = {
    0: [(0, 0, 0, 80)],
    1: [(0, 0, 80, 48), (48, 1, 0, 32)],
    2: [(0, 1, 32, 80)],
    3: [(0, 2, 0, 80)],
}


def _build_program():
    import concourse.tile as tile
    from concourse import bacc, mybir

    f32 = mybir.dt.float32
    f16 = mybir.dt.float16
    bf16 = mybir.dt.bfloat16
    fp8 = mybir.dt.float8e4
    AF = mybir.ActivationFunctionType
    ALU = mybir.AluOpType

    nc = bacc.Bacc("TRN2", target_bir_lowering=False, debug=False,
                   num_devices=N_CORES)

    stream_d = nc.dram_tensor("stream", [EMBED, SW], bf16,
                              kind="ExternalInput").ap()
    vpat_d = nc.dram_tensor("vpat", [128, VC], bf16,
                            kind="ExternalInput").ap()
    wp_d = nc.dram_tensor("wp", [128, 3 * EMBED], bf16,
                          kind="ExternalInput").ap()
    bias_d = nc.dram_tensor("biasqk", [128, 5], f32, kind="ExternalInput").ap()
    cos_d = nc.dram_tensor("cosm", [NUNITS * HD, TOK], bf16,
                           kind="ExternalInput").ap()
    sin_d = nc.dram_tensor("sinm", [NUNITS * HD, TOK], bf16,
                           kind="ExternalInput").ap()
    pit_d = nc.dram_tensor("pit", [NUNITS * HD, NUNITS * HD], fp8,
                           kind="ExternalInput").ap()
    out_d = nc.dram_tensor("outT", [EMBED, TOK], f16, kind="ExternalOutput").ap()

    def load_grouped(dst_tile, src_ap, col_w, groups, eng=None):
        for e0, e1 in groups:
            src = src_ap[128 * e0:128 * e1, :].rearrange(
                "(e p) t -> p e t", p=128)
            dst = dst_tile[:, col_w * e0:col_w * e1].rearrange(
                "p (e t) -> p e t", t=col_w)
            (eng or nc.sync).dma_start(dst, src)

    with tile.TileContext(nc) as tc:
        with tc.tile_pool(name="persist", bufs=1) as P:
            # ---- batched persistent loads, in consumption order: the whole
            # stream first (phase A is paced by it), then rotary constants,
            # then late-needed cos/sin tails and the out-proj weight ----
            str_sb = P.tile([128, KCH * SW], bf16, name="str_sb", tag="str")
            cos_sb = P.tile([128, 5 * TOK], bf16, name="cos_sb", tag="cos")
            sin_sb = P.tile([128, 5 * TOK], bf16, name="sin_sb", tag="sin")
            pit_sb = P.tile([128, 5 * NUNITS * HD], fp8, name="pit_sb",
                            tag="pit")
            bias_sb = P.tile([128, 5], f32, name="biasqk_sb", tag="biasqk")
            vpat_sb = P.tile([128, VC], bf16, name="vpat_sb", tag="vpat")
            # a single DMA transfer runs on one ring at ~70GB/s; aggregate
            # bandwidth needs PARALLEL transfers. Split the critical first
            # chunks column-wise and dispatch across four engine queues so
            # the first chunk lands ~6us instead of ~11us.
            def load_piece(e0, e1, c0, c1, eng):
                src = stream_d[128 * e0:128 * e1, c0:c1].rearrange(
                    "(e p) t -> p e t", p=128)
                dst = str_sb[:, :].rearrange(
                    "p (e t) -> p e t", t=SW)[:, e0:e1, c0:c1]
                eng.dma_start(dst, src)

            Q, H = SW // 4, SW // 2
            load_piece(0, 1, 0, Q, nc.sync)
            load_piece(0, 1, Q, 2 * Q, nc.scalar)
            load_piece(0, 1, 2 * Q, 3 * Q, nc.gpsimd)
            load_piece(0, 1, 3 * Q, SW, nc.sync)
            load_piece(1, 2, 0, H, nc.scalar)
            load_piece(1, 2, H, SW, nc.gpsimd)
            load_piece(2, 3, 0, H, nc.sync)
            load_piece(2, 3, H, SW, nc.scalar)
            load_piece(3, 4, 0, H, nc.sync)
            load_piece(3, 4, H, SW, nc.gpsimd)
            for e in (4, 5, 6, 7):
                load_piece(e, e + 1, 0, SW, nc.sync)
            load_piece(8, 10, 0, H, nc.sync)
            load_piece(8, 10, H, SW, nc.sync)
            load_grouped(pit_sb, pit_d, NUNITS * HD, [(0, 2)], eng=nc.scalar)
            load_grouped(cos_sb, cos_d, TOK, [(0, 2)], eng=nc.scalar)
            load_grouped(sin_sb, sin_d, TOK, [(0, 2)], eng=nc.scalar)
            nc.scalar.dma_start(bias_sb[:], bias_d[:])
            nc.scalar.dma_start(vpat_sb[:], vpat_d[:])
            wp_sb = P.tile([128, 3 * EMBED], bf16, name="wp_sb", tag="wp")
            # late loads (cos/sin/pit tails + wp) are not needed until after
            # stream-in; gate their transfers on stream chunk 6's arrival via
            # 1-element WAW writes into each destination so they don't steal
            # HBM bandwidth from the critical stream chunks
            LATE = [(pit_sb, pit_d, NUNITS * HD), (cos_sb, cos_d, TOK),
                    (sin_sb, sin_d, TOK)]
            gate_src = str_sb[0:1, SW * 6:SW * 6 + 1]
            for dst_t, _, cw in LATE:
                nc.vector.tensor_copy(dst_t[0:1, cw * 2:cw * 2 + 1], gate_src)
            nc.vector.tensor_copy(wp_sb[0:1, 0:1], gate_src)
            # gated dispatches ride the gpsimd queue, whose first real op
            # (the first rotary multiply) starts at stream-end anyway
            for dst_t, src_d, cw in LATE:
                load_grouped(dst_t, src_d, cw, [(2, 5)], eng=nc.gpsimd)
            nc.gpsimd.dma_start(wp_sb[:], wp_d[:])

            def xt(e):
                return str_sb[:, SW * e:SW * e + TOK]

            def wqk(e):
                o = SW * e + TOK
                return str_sb[:, o:o + NUNITS * HD]

            def wv(e):
                o = SW * e + TOK + NUNITS * HD
                return str_sb[:, o:o + VC]

            # persistent intermediates
            qkp_sb = [[None] * NSEG for _ in range(5)]
            qrot = [P.tile([HD, TOK], bf16, name=f"qrot{u}", tag=f"qrot{u}")
                    for u in range(NUNITS)]
            v_sb = [P.tile([128, VTOT], bf16, name=f"vsb{m}", tag=f"vsb{m}")
                    for m in range(TOK // 128)]
            # repacked ctx for the out-proj (head 3 of seg 1 is read directly
            # from its ctxn tile instead)
            ctxp = {(0, 0): None, (0, 1): None, (0, 2): None,
                    (1, 0): None, (1, 1): None}
            for (s_, c_) in list(ctxp):
                ctxp[(s_, c_)] = P.tile([128, SEGLEN], bf16,
                                        name=f"ctxp{c_}_{s_}",
                                        tag=f"ctxp{c_}_{s_}")
            o_sb = [P.tile([128, KCH * SEGLEN], f16, name=f"osb{s}",
                           tag=f"osb{s}") for s in range(NSEG)]
            scr = P.tile([128, 16], bf16, name="scr", tag="scr")

            # PSUM budget (8 banks): qk-proj, pi-swap and out-proj share a
            # 3-slot pool; v + 1/den broadcast 1; scores 2; ctx 2. Phase A
            # borrows all 8 for the K-outer stream consumption.
            with tc.tile_pool(name="ps_a", bufs=3, space="PSUM") as PSA, \
                 tc.tile_pool(name="ps_v", bufs=1, space="PSUM") as PSV, \
                 tc.tile_pool(name="ps_st", bufs=2, space="PSUM") as PST, \
                 tc.tile_pool(name="ps_ctx", bufs=2, space="PSUM") as PSC, \
                 tc.tile_pool(name="work", bufs=3) as W, \
                 tc.tile_pool(name="workd", bufs=6) as WD:

                ones80 = P.tile([128, HD], bf16, name="ones80", tag="ones80")
                nc.vector.memset(ones80[:], 1.0)
                # preload the ACT exp table while the engine is idle
                nc.scalar.activation(scr[:], ones80[:, 0:16], AF.Exp)
                # pad + ones columns of every v tile, once (strided memsets)
                for m_ in range(TOK // 128):
                    blk = v_sb[m_][:, :].rearrange("p (j c) -> p j c", c=VW)
                    nc.vector.memset(blk[:, :, HD:96], 0.0)
                    nc.vector.memset(blk[:, :, 96:97], 1.0)
                # zero the repack pad rows once (base partition must be
                # 32-aligned; rows 96-112 get overwritten by the j=2 piece)
                for s in range(NSEG):
                    nc.vector.memset(ctxp[(s, 1)][96:128, :], 0.0)

                est = {}    # (s, j) -> list of 4 exp'd score tiles
                ctxn_last = {}  # s -> ctxn tile of head 3 (seg-1 tail path)

                def qkproj(t, s):
                    sc = slice(SEGLEN * s, SEGLEN * (s + 1))
                    qk_ps = PSA.tile([128, SEGLEN], f32, name=f"qkps{t}_{s}",
                                     tag="mm512")
                    for e in range(KCH):
                        nc.tensor.matmul(qk_ps[:],
                                         wqk(e)[:, 128 * t:128 * (t + 1)],
                                         xt(e)[:, sc],
                                         start=(e == 0), stop=(e == KCH - 1))
                    qk_evict(t, s, qk_ps)

                def qk_evict(t, s, qk_ps):
                    q_sb = W.tile([128, SEGLEN], bf16, name=f"qsb{t}_{s}",
                                  tag="qsb", bufs=10)
                    nc.scalar.activation(q_sb[:], qk_ps[:], AF.Identity,
                                         bias=bias_sb[:, t:t + 1])
                    qkp_sb[t][s] = q_sb

                def v_evict(m, v_ps):
                    # scatter compact 80-col head blocks into the 97-stride
                    # v layout, adding the v-bias row pattern
                    dst = v_sb[m][:, :].rearrange("p (j c) -> p j c",
                                                  c=VW)[:, :, 0:HD]
                    nc.vector.scalar_tensor_tensor(
                        dst, v_ps[:].rearrange("p (j c) -> p j c", c=HD),
                        1.0, vpat_sb[:].rearrange("p (j c) -> p j c", c=HD),
                        ALU.mult, ALU.add)

                def vchunk(m, pool):
                    mc = slice(128 * m, 128 * (m + 1))
                    v_ps = pool.tile([128, VC], f32, name=f"vps{m}",
                                     tag={id(PST): "stps", id(PSC): "ctxps",
                                          id(PSV): "vps"}[id(pool)])
                    for e in range(KCH):
                        nc.tensor.matmul(v_ps[:], xt(e)[:, mc], wv(e),
                                         start=(e == 0), stop=(e == KCH - 1))
                    v_evict(m, v_ps)

                def rotary(tr, s):
                    sc = slice(SEGLEN * s, SEGLEN * (s + 1))
                    qsw_ps = PSA.tile([128, SEGLEN], f32, name=f"qsw{tr}_{s}",
                                      tag="mm512")
                    srcs = PI_BLOCKS[tr]
                    for i, tp in enumerate(srcs):
                        nc.tensor.matmul(qsw_ps[:],
                                         pit_sb[:, NUNITS * HD * tp + 128 * tr:
                                                NUNITS * HD * tp + 128 * (tr + 1)],
                                         qkp_sb[tp][s][:],
                                         start=(i == 0),
                                         stop=(i == len(srcs) - 1))
                    # t1 = q*cos on the (otherwise idle) gpsimd engine;
                    # t2 = swap(q)*sin reads PSUM so it stays on vector
                    t1 = W.tile([128, SEGLEN], bf16, name=f"t1_{tr}_{s}",
                                tag="t1", bufs=4)
                    nc.gpsimd.tensor_tensor(t1[:], qkp_sb[tr][s][:],
                                            cos_sb[:, TOK * tr + sc.start:
                                                   TOK * tr + sc.stop],
                                            ALU.mult)
                    t2 = W.tile([128, SEGLEN], bf16, name=f"t2_{tr}_{s}",
                                tag="t2", bufs=4)
                    nc.vector.tensor_tensor(t2[:], qsw_ps[:],
                                            sin_sb[:, TOK * tr + sc.start:
                                                   TOK * tr + sc.stop],
                                            ALU.mult)
                    rp = W.tile([128, SEGLEN], bf16, name=f"rotp{tr}_{s}",
                                tag="rotp", bufs=6)
                    nc.vector.tensor_tensor(rp[:], t1[:], t2[:], ALU.add)
                    for (u, po, toff, ln) in UNPACK_PIECES[tr]:
                        nc.gpsimd.dma_start(qrot[u][po:po + ln, sc],
                                            rp[toff:toff + ln, :])

                def scores(s, j):
                    sc = slice(SEGLEN * s, SEGLEN * (s + 1))
                    lst = []
                    for tkc in range(SEGLEN // 128):
                        kc = slice(SEGLEN * s + 128 * tkc,
                                   SEGLEN * s + 128 * (tkc + 1))
                        st_ps = PST.tile([128, SEGLEN], f32,
                                         name=f"st{j}_{s}_{tkc}", tag="stps")
                        nc.tensor.matmul(st_ps[:], qrot[2 * j + 1][:, kc],
                                         qrot[2 * j][:, sc],
                                         start=True, stop=True)
                        e_sb = WD.tile([128, SEGLEN], bf16,
                                       name=f"est{j}_{s}_{tkc}", tag="est",
                                       bufs=14)
                        nc.scalar.activation(e_sb[:], st_ps[:], AF.Exp)
                        lst.append(e_sb)
                    est[(s, j)] = lst

                ctx_pend = {}   # (s, j) -> (ctx_ps, rec_bf)

                def ctx_mm(s, j):
                    lst = est.pop((s, j))
                    ctx_ps = PSC.tile([128, SEGLEN], f32, name=f"ctxps{j}_{s}",
                                      tag="ctxps")
                    for tkc in range(SEGLEN // 128):
                        nc.tensor.matmul(ctx_ps[0:VW, :],
                                         v_sb[4 * s + tkc][:, VW * j:VW * (j + 1)],
                                         lst[tkc][:],
                                         start=(tkc == 0), stop=(tkc == 3))
                    # den accumulated into partition 96 (ones column of the
                    # v block). Full-width reciprocal: DVE cost is per-lane
                    # over columns so 128 partitions cost the same as 1, and
                    # the custom-DVE op only works at base partition 0; ACT
                    # Ln/Exp alternatives thrash the activation table (~1.5us
                    # ACT_TABLE_LOAD per Exp<->Ln switch, measured).
                    rec = WD.tile([128, SEGLEN], f32, name=f"rec{j}_{s}",
                                  tag="rec", bufs=2)
                    nc.vector.reciprocal_approx_fast(rec[:, :], ctx_ps[:, :])
                    rec_bf = WD.tile([128, SEGLEN], bf16,
                                     name=f"recbf{j}_{s}", tag="recbf",
                                     bufs=2)
                    nc.vector.tensor_copy(rec_bf[96:97, :], rec[96:97, :])
                    # evict the raw ctx now (releases the PSC bank; also a
                    # TensorTensor cannot read two PSUM banks at once)
                    ctx_sb = WD.tile([HD, SEGLEN], f32, name=f"ctxsb{j}_{s}",
                                     tag="ctxsb", bufs=2)
                    if j % 2 == 0:
                        nc.scalar.activation(ctx_sb[:], ctx_ps[0:HD, :],
                                             AF.Identity)
                    else:
                        nc.vector.tensor_copy(ctx_sb[:], ctx_ps[0:HD, :])
                    ctx_pend[(s, j)] = (ctx_sb, rec_bf)

                def ctx_post(s, j):
                    # emitted a beat later so the K=1 broadcast matmul (which
                    # waits on the reciprocal chain) doesn't head-of-line
                    # block ready matmuls in the PE FIFO
                    ctx_sb, rec_bf = ctx_pend.pop((s, j))
                    bc_ps = PSV.tile([HD, SEGLEN], f32, name=f"bc{j}_{s}",
                                     tag="vps")
                    nc.tensor.matmul(bc_ps[:], ones80[96:97, :],
                                     rec_bf[96:97, :], start=True, stop=True,
                                     tile_position=(96, 0))
                    ctxn = WD.tile([HD, SEGLEN], bf16, name=f"ctxn{j}_{s}",
                                   tag="ctxn", bufs=3)
                    nc.vector.tensor_tensor(ctxn[:], ctx_sb[:],
                                            bc_ps[:], ALU.mult)
                    if s == 1 and j == 3:
                        ctxn_last[s] = ctxn
                    else:
                        for (po, c, toff, ln) in CTXP_PIECES[j]:
                            nc.gpsimd.dma_start(
                                ctxp[(s, c)][toff:toff + ln, :],
                                ctxn[po:po + ln, :])

                def ctx(s, j):
                    ctx_mm(s, j)
                    ctx_post(s, j)

                def o_evict(e, s, o_ps):
                    oc = o_sb[s][:, SEGLEN * e:SEGLEN * (e + 1)]
                    if e % 2 == 0:
                        nc.vector.tensor_copy(oc, o_ps[:])
                    else:
                        nc.scalar.activation(oc, o_ps[:], AF.Identity)

                def op_c01(e, s, pool, tag):
                    o_ps = pool.tile([128, SEGLEN], f32, name=f"ops{e}_{s}",
                                     tag=tag)
                    for c in range(2):
                        nc.tensor.matmul(o_ps[:],
                                         wp_sb[:, EMBED * c + 128 * e:
                                               EMBED * c + 128 * (e + 1)],
                                         ctxp[(s, c)][:],
                                         start=(c == 0), stop=False)
                    return o_ps

                def op_c2(e, s, o_ps):
                    # K-chunk 2 is head 3 alone (80 rows); seg 1 reads the
                    # ctxn tile directly, seg 0 the repacked tile
                    rhs = (ctxn_last[1][0:HD, :] if s == 1
                           else ctxp[(0, 2)][0:HD, :])
                    nc.tensor.matmul(o_ps[0:128, :],
                                     wp_sb[0:HD, EMBED * 2 + 128 * e:
                                           EMBED * 2 + 128 * (e + 1)],
                                     rhs, start=False, stop=True)
                    o_evict(e, s, o_ps)

                def oproj(e, s):
                    o_ps = op_c01(e, s, PSA, "mm512")
                    op_c2(e, s, o_ps)

                def opair(e0, e1, s):
                    # both c01 groups first: gives the head-3 repack DMA (and
                    # the previous pair's evicts) a beat before c2 needs them
                    oa = op_c01(e0, s, PSA, "mm512")
                    ob = op_c01(e1, s, PSA, "mm512")
                    op_c2(e0, s, oa)
                    op_c2(e1, s, ob)

                def ostore(s, e0, e1):
                    src = o_sb[s][:, SEGLEN * e0:SEGLEN * e1].rearrange(
                        "p (e t) -> p e t", t=SEGLEN)
                    dst = out_d[128 * e0:128 * e1,
                                SEGLEN * s:SEGLEN * (s + 1)].rearrange(
                        "(e p) t -> p e t", p=128)
                    nc.sync.dma_start(dst, src)

                # ---- phase A: K-outer over the arriving stream. Four banks
                # accumulate seg-0's qk tiles 0-3, one K=128 slice per
                # arriving chunk; light enough that the PE tracks DMA pace
                # while cold, and the v chunks run warm in phase B ----
                qk_ko = [PSA.tile([128, SEGLEN], f32, name=f"koqk{t}",
                                  tag="mm512") for t in range(3)]
                qk_ko.append(PST.tile([128, SEGLEN], f32, name="koqk3",
                                      tag="stps"))
                s0 = slice(0, SEGLEN)
                for e in range(KCH):
                    for t in range(4):
                        nc.tensor.matmul(qk_ko[t][:],
                                         wqk(e)[:, 128 * t:128 * (t + 1)],
                                         xt(e)[:, s0],
                                         start=(e == 0), stop=(e == KCH - 1))
                for t in range(4):
                    qk_evict(t, 0, qk_ko[t])

                # ---- phase B: everything resident; seg-0 attention under
                # seg-1 projection, seg-0 out-proj inside that window ----
                vchunk(0, PST)
                vchunk(1, PSC)
                vchunk(2, PSC)
                vchunk(3, PSV)
                qkproj(4, 0)
                rotary(0, 0)
                rotary(1, 0)
                scores(0, 0)
                vchunk(4, PSC)
                ctx_mm(0, 0)
                rotary(2, 0)
                ctx_post(0, 0)
                scores(0, 1)
                vchunk(5, PSC)
                ctx_mm(0, 1)
                rotary(3, 0)
                ctx_post(0, 1)
                qkproj(0, 1)
                rotary(4, 0)
                scores(0, 2)
                vchunk(6, PST)
                ctx_mm(0, 2)
                qkproj(1, 1)
                ctx_post(0, 2)
                scores(0, 3)
                ctx_mm(0, 3)
                vchunk(7, PSV)
                ctx_post(0, 3)
                rotary(0, 1)
                qkproj(2, 1)
                opair(0, 1, 0)
                rotary(1, 1)
                scores(1, 0)
                ctx_mm(1, 0)
                qkproj(3, 1)
                ctx_post(1, 0)
                opair(2, 3, 0)
                rotary(2, 1)
                scores(1, 1)
                ctx_mm(1, 1)
                qkproj(4, 1)
                ctx_post(1, 1)
                opair(4, 5, 0)
                ostore(0, 0, 3)
                rotary(3, 1)
                rotary(4, 1)
                scores(1, 2)
                ctx_mm(1, 2)
                o6 = op_c01(6, 0, PSA, "mm512")
                ctx_post(1, 2)
                o7 = op_c01(7, 0, PSA, "mm512")
                op_c2(6, 0, o6)
                op_c2(7, 0, o7)
                opair(8, 9, 0)
                ostore(0, 3, 7)
                # ---- tail: pre-accumulate seg-1 out-proj chunks 0-1 during
                # the last attention; finish with the direct head-3 chunk ----
                scores(1, 3)
                oA = [op_c01(e, 1, PSA, "mm512") for e in range(3)]
                ctx_mm(1, 3)
                oB = [op_c01(e, 1, PST, "stps") for e in (3, 4)]
                oC = [op_c01(5, 1, PSC, "ctxps")]
                ctx_post(1, 3)
                ostore(0, 7, KCH)
                for e, o_ps in enumerate(oA + oB + oC):
                    op_c2(e, 1, o_ps)
                opair(6, 7, 1)
                ostore(1, 0, 5)
                opair(8, 9, 1)
                ostore(1, 5, KCH)

    nc.compile()
    return nc


def _prep_inputs(x, rotary_pos_emb, qkv_w, qkv_b):
    """Build per-core input shards (host-side layout/constant prep)."""
    from concourse import mybir

    x2 = np.asarray(x, np.float32).reshape(SEQ, EMBED)
    rope = np.asarray(rotary_pos_emb, np.float32)
    qkv_w = np.asarray(qkv_w, np.float32)
    qkv_b = np.asarray(qkv_b, np.float32)
    FP8 = mybir.dt.np(mybir.dt.float8e4)

    # packed rotary multipliers: packed row p = 80u + d -> r = d % 40
    r_idx = np.tile(np.arange(HD) % RH, NUNITS)      # [640]
    cos_full = np.cos(rope)[:, r_idx].T.astype(BF)   # [640, 2048]
    sin_full = np.sin(rope)[:, r_idx].T.astype(BF)

    # packed swap permutation (sign folded), block-diagonal per 80-row unit;
    # +-1 entries are exact in fp8
    D = NUNITS * HD
    Pi = np.zeros((D, D), np.float32)
    for u in range(NUNITS):
        o = HD * u
        for i in range(RH):
            Pi[o + i, o + i + RH] = -1.0
            Pi[o + i + RH, o + i] = 1.0
    pit = np.ascontiguousarray(Pi.T).astype(FP8)

    in_maps = []
    for c in range(N_CORES):
        sg, hg = divmod(c, HPC)
        toks = slice(TOK * sg, TOK * (sg + 1))
        heads = [HPC * hg + j for j in range(HPC)]

        xa = x2[toks].T                                   # [1280, 1024]

        # interleaved packing: unit 2j = q of head j, unit 2j+1 = k
        wqk = np.empty((EMBED, NUNITS * HD), np.float32)
        bias_flat = np.empty(NUNITS * HD, np.float32)
        for j, h in enumerate(heads):
            oq, ok = HD * 2 * j, HD * (2 * j + 1)
            wqk[:, oq:oq + HD] = qkv_w[HD * h:HD * (h + 1), :].T * SCALE
            bias_flat[oq:oq + HD] = qkv_b[HD * h:HD * (h + 1)] * SCALE
            ko = EMBED + HD * h
            wqk[:, ok:ok + HD] = qkv_w[ko:ko + HD, :].T
            bias_flat[ok:ok + HD] = qkv_b[ko:ko + HD]
        bias = np.ascontiguousarray(bias_flat.reshape(5, 128).T)

        # compact v weights (80 cols per head); the eviction scatters them
        # into 97-wide blocks whose col 96 is a memset ones column that makes
        # the ctx matmul accumulate the softmax denominator at partition 96
        wv = np.zeros((EMBED, VC), np.float32)
        vpat_row = np.zeros(VC, np.float32)
        for j, h in enumerate(heads):
            vo = 2 * EMBED + HD * h
            wv[:, HD * j:HD * (j + 1)] = qkv_w[vo:vo + HD, :].T
            vpat_row[HD * j:HD * (j + 1)] = qkv_b[vo:vo + HD]
        vpat = np.ascontiguousarray(np.broadcast_to(vpat_row, (128, VC)))

        stream = np.concatenate([xa, wqk, wv], axis=1)    # [1280, SW]

        # wp packed for the repacked-ctx out-proj: heads 0-2 stacked at rows
        # 0-239, head 3 alone at rows 256-335 (K-chunk 2), zeros elsewhere
        wp_cat = np.zeros((384, EMBED), np.float32)
        for j, h in enumerate(heads[:3]):
            wp_cat[HD * j:HD * (j + 1), :] = _PROJ_W[:, HD * h:HD * (h + 1)].T
        wp_cat[256:256 + HD, :] = _PROJ_W[:, HD * heads[3]:
                                          HD * (heads[3] + 1)].T
        wp = np.zeros((128, 3 * EMBED), np.float32)
        for c_ in range(3):
            wp[:, EMBED * c_:EMBED * (c_ + 1)] = wp_cat[128 * c_:128 * (c_ + 1)]

        in_maps.append({
            "stream": np.ascontiguousarray(stream).astype(BF),
            "vpat": vpat.astype(BF),
            "wp": np.ascontiguousarray(wp).astype(BF),
            "biasqk": bias,
            "cosm": np.ascontiguousarray(cos_full[:, toks]),
            "sinm": np.ascontiguousarray(sin_full[:, toks]),
            "pit": pit,
        })
    return in_maps


_PROJ_W = None


def run_on_device(inputs, trace=False, trace_cores=None):
    """Shard, run on 8 NeuronCores, gather. Returns (output, BassKernelResults)."""
    global _PROJ_W
    from concourse import bass_utils

    x = np.asarray(inputs["x"], np.float32)
    cu = np.asarray(inputs["cu_seqlens"]).tolist()
    assert cu == [0, 512, 1024, 1536, 2048], (
        f"kernel compiled for 4x512 segments, got cu_seqlens={cu}")
    assert x.shape == (SEQ, 1, EMBED)

    _PROJ_W = np.asarray(inputs["proj_w"], np.float32)
    in_maps = _prep_inputs(x, inputs["rotary_pos_emb"],
                           inputs["qkv_w"], inputs["qkv_b"])

    if "nc" not in _CACHE:
        _CACHE["nc"] = _build_program()
    nc = _CACHE["nc"]

    kw = {}
    if trace:
        kw = dict(trace=True, trace_cores=trace_cores or [0])
    res = bass_utils.run_bass_kernel_spmd(nc, in_maps,
                                          core_ids=list(range(N_CORES)), **kw)

    proj_b = np.asarray(inputs["proj_b"], np.float32)
    out = np.empty((SEQ, EMBED), np.float32)
    for sg in range(2):
        acc = res.results[HPC * sg + 0]["outT"].astype(np.float32)
        for hg in range(1, HPC):
            acc = acc + res.results[HPC * sg + hg]["outT"].astype(np.float32)
        out[TOK * sg:TOK * (sg + 1)] = acc.T
    out += proj_b
    return out.reshape(SEQ, 1, EMBED), res


def kernel(**inputs):
    out, _ = run_on_device(inputs, trace=False)
    return out
